# revision 1
# baseline (speedup 1.0000x reference)
"""Trainium2 Bass kernel for nn_CrossNetwork: 4-layer cross-network.

Reference semantics (per row b of x [B, D], D=512, L=4 layers):
    x_list = [x]
    for i in range(L):
        h = x_list[-1]
        for p in x_list[:-1]:          # sequential dot-product residuals
            s = <h_cur, p>             # scalar per row (h_cur updated each step)
            h_cur = h_cur + s * ones
        y = h_cur @ W[i].T + b[i]
        x_list.append(y)
    out = concat(x_list[1:])           # [B, L*D]

Key algebraic restructure (exact): adding a per-row scalar s to every
component only shifts later dot products by s * rowsum(prior).  With
D_j = <h, p_j> (h = the layer input, unmodified) and sig_j = rowsum(p_j):
    s'_j = D_j + S_{<j} * sig_j ;  S = sum_j s'_j
so only the plain dots D_j, the row-sums sig_j of y0/y1, and a tiny
per-row recurrence are needed; the shift S is applied once per layer.

Layout: batch rows on SBUF partitions ([128, 512] tiles), activations f32.
Matmul stationary = PE-transposed activation chunks; moving = host-
pre-transposed W^T.  Bias via an extra K=1 accumulating matmul.
Sharding: batch split across 8 NeuronCores (data parallel, SPMD).
"""

import numpy as np

NUM_LAYERS = 4
D = 512
B = 16384
N_CORES = 8
ROWS_PER_CORE = B // N_CORES          # 2048
NTILES = ROWS_PER_CORE // 128         # 16
NCH = D // 128                        # 4 contraction chunks

# matmul operand dtype: "bf16" or "f32r"
MM_DTYPE = "f32r"
# row-dot reduction: "ts_accum" (mul + tensor_scalar reduce) or
# "reduce" (mul + tensor_reduce)
DOT_MODE = "ts_accum"

_CACHE = {}


def _build_nc(ntiles=NTILES):
    import concourse.tile as tile
    from concourse import bacc, mybir
    from concourse.masks import make_identity

    F32 = mybir.dt.float32
    BF16 = mybir.dt.bfloat16
    F32R = mybir.dt.float32r
    AF = mybir.ActivationFunctionType
    MUL = mybir.AluOpType.mult
    ADD = mybir.AluOpType.add

    MMDT = F32R if MM_DTYPE == "f32r" else BF16
    FINDT = F32 if MM_DTYPE == "f32r" else BF16
    rows = ntiles * 128

    nc = bacc.Bacc("TRN2", target_bir_lowering=False, debug=False)

    X = nc.dram_tensor("x", [rows, D], F32, kind="ExternalInput")
    WT = nc.dram_tensor("wt", [NUM_LAYERS, D, D], MMDT, kind="ExternalInput")
    BIAS = nc.dram_tensor("bias", [NUM_LAYERS, D], MMDT, kind="ExternalInput")
    OUT = nc.dram_tensor("out", [rows, NUM_LAYERS * D], F32,
                         kind="ExternalOutput")

    with tile.TileContext(nc) as tc:
        with (
            tc.tile_pool(name="consts", bufs=1) as consts,
            tc.tile_pool(name="acts", bufs=2) as acts,
            tc.tile_pool(name="fins", bufs=3) as fins,
            tc.tile_pool(name="scratch", bufs=2) as scratch,
            tc.tile_pool(name="scals", bufs=2) as scals,
            tc.tile_pool(name="ypsum", bufs=3, space="PSUM") as ypsum,
            tc.tile_pool(name="tpsum", bufs=3, space="PSUM") as tpsum,
        ):
            # ---- constants (loaded once) ----
            wt_sb = consts.tile([128, NUM_LAYERS, NCH, D], MMDT)
            wt_dram = WT.rearrange("l (c p) e -> l c p e", p=128)
            for i in range(NUM_LAYERS):
                for c in range(NCH):
                    nc.sync.dma_start(wt_sb[:, i, c, :], wt_dram[i, c, :, :])
            bias_sb = consts.tile([1, NUM_LAYERS, D], MMDT)
            for i in range(NUM_LAYERS):
                nc.sync.dma_start(bias_sb[0:1, i, :], BIAS[i:i + 1, :])
            ones_f32 = consts.tile([1, 128], F32)
            nc.vector.memset(ones_f32[:], 1.0)
            ones_row = consts.tile([1, 128], MMDT)
            nc.vector.tensor_copy(ones_row[:], ones_f32[:])
            ident = consts.tile([128, 128], FINDT)
            make_identity(nc, ident[:])

            x_dram = X.rearrange("(t p) d -> t p d", p=128)
            out_dram = OUT.rearrange("(t p) d -> t p d", p=128)

            def row_reduce(src_ap, dst_col, tag):
                """dst_col[128,1] = rowsum(src_ap [128,D])."""
                if DOT_MODE == "ts_accum":
                    waste = scratch.tile([128, D], F32, tag=tag)
                    nc.vector.tensor_scalar(
                        out=waste[:], in0=src_ap, scalar1=0.0, scalar2=None,
                        op0=ADD, op1=ADD, accum_out=dst_col)
                else:
                    nc.vector.tensor_reduce(
                        out=dst_col, in_=src_ap, op=ADD,
                        axis=mybir.AxisListType.X)

            for t in range(ntiles):
                # ---- load x tile ----
                x_t = acts.tile([128, D], F32, tag="x")
                nc.sync.dma_start(x_t[:], x_dram[t, :, :])

                ys = []      # f32 activation tiles [x_t, y0, y1, y2]
                sigs = {}    # rowsum columns for y0, y1

                scal = scals.tile([128, 16], F32, tag="scal")
                ncol = [0]
                def col():
                    c = ncol[0]; ncol[0] += 1
                    return scal[:, c:c + 1]

                h = x_t
                ys.append(x_t)

                for i in range(NUM_LAYERS):
                    # ---- dots vs priors + recurrence -> S (skip layer 0) ----
                    S = None
                    if i >= 1:
                        Ds = []
                        for j, p in enumerate(ys[:-1]):
                            prod = scratch.tile([128, D], F32, tag="prod")
                            nc.vector.tensor_tensor(
                                out=prod[:], in0=h[:], in1=p[:], op=MUL)
                            Dj = col()
                            row_reduce(prod[:], Dj, "dotred")
                            Ds.append(Dj)
                        if i == 1:
                            S = Ds[0]
                        elif i == 2:
                            # S = D0 + D1 + D0*sig(y0)
                            u = col()
                            nc.vector.tensor_scalar(
                                out=u, in0=sigs[0], scalar1=Ds[0], scalar2=Ds[0],
                                op0=MUL, op1=ADD)  # u = sig0*D0 + D0
                            S = col()
                            nc.vector.tensor_scalar(
                                out=S, in0=u, scalar1=Ds[1], scalar2=None, op0=ADD)
                        else:
                            # priors x, y0, y1 with sig(y0), sig(y1)
                            u = col()
                            nc.vector.tensor_scalar(
                                out=u, in0=sigs[0], scalar1=Ds[0], scalar2=Ds[0],
                                op0=MUL, op1=ADD)          # u = D0*(1+sig0)
                            sa = col()
                            nc.vector.tensor_scalar(
                                out=sa, in0=u, scalar1=Ds[1], scalar2=None, op0=ADD)
                            v = col()
                            nc.vector.tensor_scalar(
                                out=v, in0=sigs[1], scalar1=sa, scalar2=sa,
                                op0=MUL, op1=ADD)          # v = sa*(1+sig1)
                            S = col()
                            nc.vector.tensor_scalar(
                                out=S, in0=v, scalar1=Ds[2], scalar2=None, op0=ADD)

                    # ---- x_fin = h + S (gpsimd; casts when FINDT != F32) ----
                    if S is None:
                        if FINDT == F32:
                            x_fin = h
                        else:
                            x_fin = fins.tile([128, D], FINDT, tag="fin")
                            nc.gpsimd.tensor_copy(x_fin[:], h[:])
                    else:
                        x_fin = fins.tile([128, D], FINDT, tag="fin")
                        nc.gpsimd.tensor_scalar_add(x_fin[:], h[:], S)

                    # ---- transpose x_fin -> stationary chunks ----
                    tr = tpsum.tile([128, NCH, 128], FINDT, tag="tr")
                    for c in range(NCH):
                        nc.tensor.transpose(
                            tr[:, c, :], x_fin[:, c * 128:(c + 1) * 128], ident[:])
                    xT = fins.tile([128, NCH, 128], MMDT, tag="xT")
                    nc.scalar.activation(xT[:], tr[:], AF.Copy)

                    # ---- matmuls: y = x_fin @ W_i^T + bias ----
                    y_ps = ypsum.tile([128, D], F32, tag="y")
                    for c in range(NCH):
                        nc.tensor.matmul(
                            y_ps[:], xT[:, c, :], wt_sb[:, i, c, :],
                            start=(c == 0), stop=False)
                    nc.tensor.matmul(
                        y_ps[:], ones_row[:], bias_sb[:, i, :],
                        start=False, stop=True)

                    # ---- P1: copy y psum -> sbuf f32; sigma for y0, y1 ----
                    y = acts.tile([128, D], F32, tag=f"y{i}")
                    nc.scalar.activation(y[:], y_ps[:], AF.Copy)
                    if i in (0, 1):
                        sig = col()
                        row_reduce(y[:], sig, "sigred")
                        sigs[i] = sig

                    # ---- DMA out ----
                    nc.sync.dma_start(out_dram[t, :, i * D:(i + 1) * D], y[:])

                    ys.append(y)
                    h = y

    nc.compile()
    return nc


def _host_prep(W, b):
    """W [L,D,D] f32 (torch Linear layout: y = x @ W.T) -> transposed WT[l,d,e]."""
    WT = np.ascontiguousarray(W.transpose(0, 2, 1))
    bias = np.ascontiguousarray(b)
    if MM_DTYPE == "f32r":
        # PE accepts raw f32 bits for f32r DRAM operands (verified on HW:
        # identical error to DVE-rounded) — no host rounding needed.
        return WT, bias
    else:
        import ml_dtypes
        return (np.asarray(WT, dtype=ml_dtypes.bfloat16),
                np.asarray(bias, dtype=ml_dtypes.bfloat16))


def run_shards(x, W, b, **spmd_kwargs):
    """Run the SPMD kernel; returns (full_output, BassKernelResults)."""
    from concourse.bass_utils import run_bass_kernel_spmd

    x = np.ascontiguousarray(np.asarray(x, np.float32))
    WT, bias = _host_prep(np.asarray(W, np.float32), np.asarray(b, np.float32))

    if "nc" not in _CACHE:
        _CACHE["nc"] = _build_nc()
    nc = _CACHE["nc"]

    in_maps = []
    for c in range(N_CORES):
        shard = x[c * ROWS_PER_CORE:(c + 1) * ROWS_PER_CORE]
        in_maps.append({"x": np.ascontiguousarray(shard), "wt": WT, "bias": bias})

    res = run_bass_kernel_spmd(nc, in_maps, core_ids=list(range(N_CORES)),
                               **spmd_kwargs)
    out = np.concatenate([r["out"] for r in res.results], axis=0)
    return out.astype(np.float32), res


def kernel(x, W, b):
    out, _ = run_shards(x, W, b)
    return out



# revision 20
# speedup vs baseline: 4.7302x; 4.7302x over previous
"""Trainium2 Bass kernel for nn_CrossNetwork: 4-layer cross-network.

Reference semantics (per row b of x [B, D], D=512, L=4):
    x_list = [x]
    for i in range(L):
        h = x_list[-1]
        for p in x_list[:-1]:
            s = <h_cur, p>; h_cur += s        # sequential scalar residuals
        y = h_cur @ W[i].T + b[i]
        x_list.append(y)
    out = concat(x_list[1:])

Restructure (exact algebra):
 1. Scalar-shift recurrence: with D_j = <h, p_j> (h = raw layer input) and
    sig_j = rowsum(p_j):  S = sum_j s'_j,  s'_j = D_j + S_{<j} * sig_j.
 2. Bias elimination: only bias-free h'_i = x_fin'_i @ Wt_i live on chip,
    x_fin'_i = h'_{i-1} + S_i; the true y_i = h'_i + c_i with fixed
    c_i = c_{i-1} @ Wt_i + b_i (host adds c_i to the output).  All cross
    terms <h'_a, c_m> / rowsum(h'_a) are matmuls against fixed vectors ->
    a few extra moving columns sharing the main matmul's stationary.

Schedule: layer-major over 8 macro-tiles of 256 rows (2 subtiles x 128
partitions).  All per-row-scalar algebra is batched into [128, 16] ops
(one column per subtile).  Dots: one bf16 tensor_tensor multiply per
(layer, macro-tile) with the h operand broadcast over priors + one
segmented tensor_reduce (axis=X).  On-chip dtype bf16 except PSUM / the
per-row scalars (f32).  Output written bf16; host upcasts and adds c_i.
Sharding: batch split across 8 NeuronCores (data parallel, SPMD).
"""

import os
import numpy as np

NUM_LAYERS = 4
D = 512
B = 16384
N_CORES = 8
ROWS_PER_CORE = B // N_CORES          # 2048
ST = 2                                # subtiles per macro-tile
NMT = ROWS_PER_CORE // (128 * ST)     # 8 macro-tiles
NCH = D // 128                        # 4 contraction chunks

N_EX = (6, 3, 2, 0)                   # extras columns per layer
EX_OFF = (0, 6, 9, 11)
N_EX_TOT = 11

SHIFT_ACT = bool(int(os.environ.get("K_SHIFT_ACT", "1")))   # shifts on ACT
TRANSP = os.environ.get("K_TRANSP", "xbar")  # xbar | pe_act | pe_dve
RED_BF16 = bool(int(os.environ.get("K_RED_BF16", "1")))     # reduce out bf16

_CACHE = {}


def _build_nc(nmt=NMT, consts=None):
    import concourse.tile as tile
    from concourse import bacc, mybir
    from concourse.masks import make_identity

    F32 = mybir.dt.float32
    BF16 = mybir.dt.bfloat16
    AF = mybir.ActivationFunctionType
    MUL = mybir.AluOpType.mult
    ADD = mybir.AluOpType.add

    csum0, csum1, K10, K20, K21 = consts
    rows = nmt * ST * 128
    nsub = nmt * ST                       # subtile count (= scalar columns)
    RDT = BF16 if RED_BF16 else F32

    nc = bacc.Bacc("TRN2", target_bir_lowering=False, debug=False)

    X = nc.dram_tensor("x", [rows, D], BF16, kind="ExternalInput")
    WT = nc.dram_tensor("wt", [NUM_LAYERS, D, D], BF16, kind="ExternalInput")
    EX = nc.dram_tensor("ex", [D, N_EX_TOT], BF16, kind="ExternalInput")
    OUT = nc.dram_tensor("out", [rows, NUM_LAYERS * D], BF16,
                         kind="ExternalOutput")

    with tile.TileContext(nc) as tc:
        with (
            tc.tile_pool(name="consts", bufs=1) as cpool,
            tc.tile_pool(name="persist", bufs=1) as ppool,
            tc.tile_pool(name="fins", bufs=3) as fins,
            tc.tile_pool(name="xts", bufs=3) as xts,
            tc.tile_pool(name="waste", bufs=2) as waste,
            tc.tile_pool(name="dcols", bufs=2) as dpool,
            tc.tile_pool(name="tmps", bufs=24) as tmps,
            tc.tile_pool(name="ypsum", bufs=2, space="PSUM") as ypsum,
            tc.tile_pool(name="tpsum", bufs=2, space="PSUM") as tpsum,
            tc.tile_pool(name="epsum", bufs=2, space="PSUM") as epsum,
        ):
            # ---- constants ----
            wt_sb = cpool.tile([128, NUM_LAYERS, NCH, D], BF16)
            wt_dram = WT.rearrange("l (c p) e -> l c p e", p=128)
            for i in range(NUM_LAYERS):
                for c in range(NCH):
                    nc.sync.dma_start(wt_sb[:, i, c, :], wt_dram[i, c, :, :])
            ex_sb = cpool.tile([128, NCH, N_EX_TOT], BF16)
            ex_dram = EX.rearrange("(c p) n -> c p n", p=128)
            for c in range(NCH):
                nc.sync.dma_start(ex_sb[:, c, :], ex_dram[c, :, :])
            ident = cpool.tile([128, 128], BF16)
            make_identity(nc, ident[:])
            # batched per-row scalars: one column per subtile
            Ecols = cpool.tile([128, N_EX_TOT, nsub], F32)
            Scols = cpool.tile([128, NUM_LAYERS, nsub], F32)

            x_dram = X.rearrange("(m s p) d -> m s p d", s=ST, p=128)
            out_dram = OUT.rearrange("(m s p) d -> m s p d", s=ST, p=128)

            # persistent activation slots: 0 = x, 1+i = h'_i
            xh = [ppool.tile([128, NUM_LAYERS + 1, ST, D], BF16, tag=f"xh{m}",
                             name=f"xh{m}")
                  for m in range(nmt)]
            for m in range(nmt):
                for s in range(ST):
                    nc.sync.dma_start(xh[m][:, 0, s, :], x_dram[m, s, :, :])

            def tt(name, in0, in1, op):
                o = tmps.tile([128, nsub], F32, tag="tmp")
                nc.vector.tensor_tensor(out=o[:], in0=in0, in1=in1, op=op)
                return o[:]

            def ts_imm(name, in0, imm, op):
                o = tmps.tile([128, nsub], F32, tag="tmp")
                nc.vector.tensor_scalar(out=o[:], in0=in0, scalar1=float(imm),
                                        scalar2=None, op0=op)
                return o[:]

            for i in range(NUM_LAYERS):
                # ---- dots + batched recurrence -> S_i (skip layer 0) ----
                if i > 0:
                    Dc = dpool.tile([128, i, nsub], F32, tag="D")
                    for m in range(nmt):
                        w = waste.tile([128, i, ST, D], BF16, tag="w")
                        h_b = xh[m][:, i:i + 1, :, :].broadcast_to(
                            (128, i, ST, D))
                        nc.vector.tensor_tensor(
                            out=w[:], in0=h_b, in1=xh[m][:, 0:i, :, :],
                            op=MUL)
                        if RED_BF16:
                            rtmp = waste.tile([128, i, ST], BF16, tag="r")
                            with nc.allow_low_precision("bf16 dot store"):
                                nc.vector.tensor_reduce(
                                    out=rtmp[:], in_=w[:],
                                    axis=mybir.AxisListType.X, op=ADD)
                            nc.vector.tensor_copy(
                                Dc[:, :, m * ST:(m + 1) * ST], rtmp[:])
                        else:
                            nc.vector.tensor_reduce(
                                out=Dc[:, :, m * ST:(m + 1) * ST], in_=w[:],
                                axis=mybir.AxisListType.X, op=ADD)
                    E = lambda k: Ecols[:, k, :]
                    Dr = lambda j: Dc[:, j, :]
                    if i == 1:
                        nc.vector.tensor_tensor(
                            out=Scols[:, 1, :], in0=Dr(0), in1=E(0), op=ADD)
                    elif i == 2:
                        Da = tt("Da", Dr(0), E(1), ADD)
                        pre = tt("pre", E(6), E(3), ADD)
                        pre = ts_imm("preK", pre, K10, ADD)
                        Db = tt("Db", Dr(1), pre, ADD)
                        ta = ts_imm("ta", Da, 1.0 + csum0, MUL)
                        ta = tt("ta2", ta, Db, ADD)
                        t3 = tt("t3", E(5), Da, MUL)
                        nc.vector.tensor_tensor(
                            out=Scols[:, 2, :], in0=t3, in1=ta, op=ADD)
                    else:
                        Da = tt("Da", Dr(0), E(2), ADD)
                        p1 = tt("p1", E(9), E(4), ADD)
                        p1 = ts_imm("p1K", p1, K20, ADD)
                        Db = tt("Db", Dr(1), p1, ADD)
                        p2 = tt("p2", E(10), E(7), ADD)
                        p2 = ts_imm("p2K", p2, K21, ADD)
                        Dck = tt("Dck", Dr(2), p2, ADD)
                        ta = ts_imm("ta", Da, 1.0 + csum0, MUL)
                        ta = tt("ta2", ta, Db, ADD)
                        t3 = tt("t3", E(5), Da, MUL)
                        b_ = tt("b", t3, ta, ADD)
                        tcx = ts_imm("tc", b_, 1.0 + csum1, MUL)
                        tcx = tt("tc2", tcx, Dck, ADD)
                        t4 = tt("t4", E(8), b_, MUL)
                        nc.vector.tensor_tensor(
                            out=Scols[:, 3, :], in0=t4, in1=tcx, op=ADD)

                for m in range(nmt):
                    # ---- x_fin' = h'_{i-1} + S_i (per subtile) ----
                    if i == 0:
                        x_fin = xh[m][:, 0, :, :]
                    else:
                        xf = fins.tile([128, ST, D], BF16, tag="fin")
                        for s in range(ST):
                            scol = Scols[:, i, m * ST + s:m * ST + s + 1]
                            if SHIFT_ACT:
                                nc.scalar.activation(
                                    xf[:, s, :], xh[m][:, i, s, :],
                                    AF.Identity, bias=scol, scale=1.0)
                            else:
                                nc.vector.tensor_scalar(
                                    out=xf[:, s, :], in0=xh[m][:, i, s, :],
                                    scalar1=scol, scalar2=None, op0=ADD)
                        x_fin = xf[:]

                    # ---- transposes -> xT (stationary operands) ----
                    xT = xts.tile([128, ST, NCH, 128], BF16, tag="xT")
                    if TRANSP == "xbar":
                        for s in range(ST):
                            for c in range(NCH):
                                nc.sync.dma_start_transpose(
                                    xT[:, s, c, :],
                                    x_fin[:, s, c * 128:(c + 1) * 128])
                    else:
                        tr = tpsum.tile([128, ST, NCH, 128], BF16, tag="tr")
                        for s in range(ST):
                            for c in range(NCH):
                                nc.tensor.transpose(
                                    tr[:, s, c, :],
                                    x_fin[:, s, c * 128:(c + 1) * 128],
                                    ident[:])
                        if TRANSP == "pe_dve":
                            nc.vector.tensor_copy(xT[:], tr[:])
                        else:
                            nc.scalar.activation(xT[:], tr[:], AF.Copy)

                    # ---- matmuls ----
                    y_ps = ypsum.tile([128, ST, D], F32, tag="y")
                    nex = N_EX[i]
                    if nex:
                        e_ps = epsum.tile([128, ST, 16], F32, tag="e")
                    for s in range(ST):
                        for c in range(NCH):
                            nc.tensor.matmul(
                                y_ps[:, s, :], xT[:, s, c, :],
                                wt_sb[:, i, c, :],
                                start=(c == 0), stop=(c == NCH - 1))
                            if nex:
                                nc.tensor.matmul(
                                    e_ps[:, s, :nex], xT[:, s, c, :],
                                    ex_sb[:, c, EX_OFF[i]:EX_OFF[i] + nex],
                                    start=(c == 0), stop=(c == NCH - 1))

                    # ---- evictions ----
                    nc.scalar.activation(xh[m][:, i + 1, :, :], y_ps[:],
                                         AF.Copy)
                    if nex:
                        nc.scalar.activation(
                            Ecols[:, EX_OFF[i]:EX_OFF[i] + nex,
                                  m * ST:(m + 1) * ST],
                            e_ps[:, :, :nex].rearrange("p s k -> p k s"),
                            AF.Copy)

            # ---- DMA out ----
            out_dram_ld = OUT.rearrange("(m s p) (l d) -> m s p l d",
                                        s=ST, p=128, l=NUM_LAYERS)
            for m in range(nmt):
                for s in range(ST):
                    nc.sync.dma_start(
                        out_dram_ld[m, s, :, :, :], xh[m][:, 1:, s, :])

    nc.compile()
    return nc


def _host_prep(W, b):
    """Fold biases into fixed vectors; build bf16 weight/extras operands."""
    import ml_dtypes
    W64 = np.asarray(W, np.float64)
    b64 = np.asarray(b, np.float64)
    Wt = W64.transpose(0, 2, 1)                      # [L, d, e]: y = x@Wt + b
    ones = np.ones(D)
    c = []
    prev = np.zeros(D)
    for i in range(NUM_LAYERS):
        ci = prev @ Wt[i] + b64[i]
        c.append(ci)
        prev = ci
    ex_cols = [c[0], c[1], c[2], Wt[0] @ c[1], Wt[0] @ c[2], Wt[0] @ ones,
               Wt[1] @ c[0], Wt[1] @ c[2], Wt[1] @ ones,
               Wt[2] @ c[0], Wt[2] @ c[1]]
    ex = np.stack(ex_cols, axis=1)                   # [512, 11]
    consts = (float(np.sum(c[0])), float(np.sum(c[1])),
              float(c[1] @ c[0]), float(c[2] @ c[0]), float(c[2] @ c[1]))
    bf = ml_dtypes.bfloat16
    wt_bf = np.ascontiguousarray(Wt.astype(np.float32)).astype(bf)
    ex_bf = np.ascontiguousarray(ex.astype(np.float32)).astype(bf)
    c_f32 = np.stack([ci.astype(np.float32) for ci in c], axis=0)  # [L, 512]
    return wt_bf, ex_bf, consts, c_f32


def run_shards(x, W, b, **spmd_kwargs):
    """Run the SPMD kernel; returns (full_output, BassKernelResults)."""
    import ml_dtypes
    from concourse.bass_utils import run_bass_kernel_spmd

    x_bf = np.ascontiguousarray(
        np.asarray(x, np.float32).astype(ml_dtypes.bfloat16))
    wt_bf, ex_bf, consts, c_f32 = _host_prep(np.asarray(W, np.float32),
                                             np.asarray(b, np.float32))

    if "nc" not in _CACHE:
        _CACHE["nc"] = _build_nc(consts=consts)
    nc = _CACHE["nc"]

    in_maps = []
    for cid in range(N_CORES):
        shard = x_bf[cid * ROWS_PER_CORE:(cid + 1) * ROWS_PER_CORE]
        in_maps.append({"x": np.ascontiguousarray(shard),
                        "wt": wt_bf, "ex": ex_bf})

    res = run_bass_kernel_spmd(nc, in_maps, core_ids=list(range(N_CORES)),
                               **spmd_kwargs)
    out = np.concatenate([r["out"] for r in res.results], axis=0)
    out = out.astype(np.float32)
    out += np.reshape(c_f32, (1, NUM_LAYERS * D))     # host bias fold-back
    return out, res


def kernel(x, W, b):
    out, _ = run_shards(x, W, b)
    return out


# revision 23
# speedup vs baseline: 4.7941x; 1.0135x over previous
"""Trainium2 Bass kernel for nn_CrossNetwork: 4-layer cross-network.

Reference semantics (per row b of x [B, D], D=512, L=4):
    x_list = [x]
    for i in range(L):
        h = x_list[-1]
        for p in x_list[:-1]:
            s = <h_cur, p>; h_cur += s        # sequential scalar residuals
        y = h_cur @ W[i].T + b[i]
        x_list.append(y)
    out = concat(x_list[1:])

Restructure (exact algebra):
 1. Scalar-shift recurrence: with D_j = <h, p_j> (h = raw layer input) and
    sig_j = rowsum(p_j):  S = sum_j s'_j,  s'_j = D_j + S_{<j} * sig_j.
 2. Bias elimination: only bias-free h'_i = x_fin'_i @ Wt_i live on chip,
    x_fin'_i = h'_{i-1} + S_i; the true y_i = h'_i + c_i with fixed
    c_i = c_{i-1} @ Wt_i + b_i (host adds c_i to the output).  All cross
    terms <h'_a, c_m> / rowsum(h'_a) are matmuls against fixed vectors ->
    a few extra moving columns sharing the main matmul's stationary.

Schedule: layer-major over 8 macro-tiles of 256 rows (2 subtiles x 128
partitions).  All per-row-scalar algebra is batched into [128, 16] ops
(one column per subtile).  Dots: one bf16 tensor_tensor multiply per
(layer, macro-tile) with the h operand broadcast over priors + one
segmented tensor_reduce (axis=X).  On-chip dtype bf16 except PSUM / the
per-row scalars (f32).  Output written bf16; host upcasts and adds c_i.
Sharding: batch split across 8 NeuronCores (data parallel, SPMD).
"""

import os
import numpy as np

NUM_LAYERS = 4
D = 512
B = 16384
N_CORES = 8
ROWS_PER_CORE = B // N_CORES          # 2048
ST = 2                                # subtiles per macro-tile
NMT = ROWS_PER_CORE // (128 * ST)     # 8 macro-tiles
NCH = D // 128                        # 4 contraction chunks

N_EX = (6, 3, 2, 0)                   # extras columns per layer
EX_OFF = (0, 6, 9, 11)
N_EX_TOT = 11

SHIFT_ACT = bool(int(os.environ.get("K_SHIFT_ACT", "1")))   # shifts on ACT
TRANSP = os.environ.get("K_TRANSP", "pe_dve")  # xbar | pe_act | pe_dve
RED_BF16 = bool(int(os.environ.get("K_RED_BF16", "0")))     # reduce out bf16
RED_MODE = os.environ.get("K_RED_MODE", "tr")  # tr | tsacc

_CACHE = {}


def _build_nc(nmt=NMT, consts=None):
    import concourse.tile as tile
    from concourse import bacc, mybir
    from concourse.masks import make_identity

    F32 = mybir.dt.float32
    BF16 = mybir.dt.bfloat16
    AF = mybir.ActivationFunctionType
    MUL = mybir.AluOpType.mult
    ADD = mybir.AluOpType.add

    csum0, csum1, K10, K20, K21 = consts
    rows = nmt * ST * 128
    nsub = nmt * ST                       # subtile count (= scalar columns)
    RDT = BF16 if RED_BF16 else F32

    nc = bacc.Bacc("TRN2", target_bir_lowering=False, debug=False)

    X = nc.dram_tensor("x", [rows, D], BF16, kind="ExternalInput")
    WT = nc.dram_tensor("wt", [NUM_LAYERS, D, D], BF16, kind="ExternalInput")
    EX = nc.dram_tensor("ex", [D, N_EX_TOT], BF16, kind="ExternalInput")
    OUT = nc.dram_tensor("out", [rows, NUM_LAYERS * D], BF16,
                         kind="ExternalOutput")

    with tile.TileContext(nc) as tc:
        with (
            tc.tile_pool(name="consts", bufs=1) as cpool,
            tc.tile_pool(name="persist", bufs=1) as ppool,
            tc.tile_pool(name="fins", bufs=3) as fins,
            tc.tile_pool(name="xts", bufs=3) as xts,
            tc.tile_pool(name="waste", bufs=2) as waste,
            tc.tile_pool(name="dcols", bufs=2) as dpool,
            tc.tile_pool(name="tmps", bufs=24) as tmps,
            tc.tile_pool(name="ypsum", bufs=2, space="PSUM") as ypsum,
            tc.tile_pool(name="tpsum", bufs=2, space="PSUM") as tpsum,
            tc.tile_pool(name="epsum", bufs=2, space="PSUM") as epsum,
        ):
            # ---- constants ----
            wt_sb = cpool.tile([128, NUM_LAYERS, NCH, D], BF16)
            wt_dram = WT.rearrange("l (c p) e -> l c p e", p=128)
            for i in range(NUM_LAYERS):
                for c in range(NCH):
                    nc.sync.dma_start(wt_sb[:, i, c, :], wt_dram[i, c, :, :])
            ex_sb = cpool.tile([128, NCH, N_EX_TOT], BF16)
            ex_dram = EX.rearrange("(c p) n -> c p n", p=128)
            for c in range(NCH):
                nc.sync.dma_start(ex_sb[:, c, :], ex_dram[c, :, :])
            ident = cpool.tile([128, 128], BF16)
            make_identity(nc, ident[:])
            # batched per-row scalars: one column per subtile
            Ecols = cpool.tile([128, N_EX_TOT, nsub], F32)
            Scols = cpool.tile([128, NUM_LAYERS, nsub], F32)

            x_dram = X.rearrange("(m s p) d -> m s p d", s=ST, p=128)
            out_dram = OUT.rearrange("(m s p) d -> m s p d", s=ST, p=128)

            # persistent activation slots: 0 = x, 1+i = h'_i
            xh = [ppool.tile([128, NUM_LAYERS + 1, ST, D], BF16, tag=f"xh{m}",
                             name=f"xh{m}")
                  for m in range(nmt)]
            for m in range(nmt):
                for s in range(ST):
                    nc.sync.dma_start(xh[m][:, 0, s, :], x_dram[m, s, :, :])

            def tt(name, in0, in1, op):
                o = tmps.tile([128, nsub], F32, tag="tmp")
                nc.vector.tensor_tensor(out=o[:], in0=in0, in1=in1, op=op)
                return o[:]

            def ts_imm(name, in0, imm, op):
                o = tmps.tile([128, nsub], F32, tag="tmp")
                nc.vector.tensor_scalar(out=o[:], in0=in0, scalar1=float(imm),
                                        scalar2=None, op0=op)
                return o[:]

            for i in range(NUM_LAYERS):
                # ---- dots + batched recurrence -> S_i (skip layer 0) ----
                if i > 0:
                    Dc = dpool.tile([128, i, nsub], F32, tag="D")
                    for m in range(nmt):
                        w = waste.tile([128, i, ST, D], BF16, tag="w")
                        h_b = xh[m][:, i:i + 1, :, :].broadcast_to(
                            (128, i, ST, D))
                        nc.vector.tensor_tensor(
                            out=w[:], in0=h_b, in1=xh[m][:, 0:i, :, :],
                            op=MUL)
                        if RED_MODE == "tsacc":
                            wf = waste.tile([128, i, ST, D], BF16, tag="wf")
                            for j in range(i):
                                for s in range(ST):
                                    nc.vector.tensor_scalar(
                                        out=wf[:, j, s, :], in0=w[:, j, s, :],
                                        scalar1=0.0, scalar2=None,
                                        op0=ADD, op1=ADD,
                                        accum_out=Dc[:, j, m * ST + s:
                                                     m * ST + s + 1])
                        elif RED_BF16:
                            rtmp = waste.tile([128, i, ST], BF16, tag="r")
                            with nc.allow_low_precision("bf16 dot store"):
                                nc.vector.tensor_reduce(
                                    out=rtmp[:], in_=w[:],
                                    axis=mybir.AxisListType.X, op=ADD)
                            nc.vector.tensor_copy(
                                Dc[:, :, m * ST:(m + 1) * ST], rtmp[:])
                        else:
                            nc.vector.tensor_reduce(
                                out=Dc[:, :, m * ST:(m + 1) * ST], in_=w[:],
                                axis=mybir.AxisListType.X, op=ADD)
                    E = lambda k: Ecols[:, k, :]
                    Dr = lambda j: Dc[:, j, :]
                    if i == 1:
                        nc.vector.tensor_tensor(
                            out=Scols[:, 1, :], in0=Dr(0), in1=E(0), op=ADD)
                    elif i == 2:
                        Da = tt("Da", Dr(0), E(1), ADD)
                        pre = tt("pre", E(6), E(3), ADD)
                        pre = ts_imm("preK", pre, K10, ADD)
                        Db = tt("Db", Dr(1), pre, ADD)
                        ta = ts_imm("ta", Da, 1.0 + csum0, MUL)
                        ta = tt("ta2", ta, Db, ADD)
                        t3 = tt("t3", E(5), Da, MUL)
                        nc.vector.tensor_tensor(
                            out=Scols[:, 2, :], in0=t3, in1=ta, op=ADD)
                    else:
                        Da = tt("Da", Dr(0), E(2), ADD)
                        p1 = tt("p1", E(9), E(4), ADD)
                        p1 = ts_imm("p1K", p1, K20, ADD)
                        Db = tt("Db", Dr(1), p1, ADD)
                        p2 = tt("p2", E(10), E(7), ADD)
                        p2 = ts_imm("p2K", p2, K21, ADD)
                        Dck = tt("Dck", Dr(2), p2, ADD)
                        ta = ts_imm("ta", Da, 1.0 + csum0, MUL)
                        ta = tt("ta2", ta, Db, ADD)
                        t3 = tt("t3", E(5), Da, MUL)
                        b_ = tt("b", t3, ta, ADD)
                        tcx = ts_imm("tc", b_, 1.0 + csum1, MUL)
                        tcx = tt("tc2", tcx, Dck, ADD)
                        t4 = tt("t4", E(8), b_, MUL)
                        nc.vector.tensor_tensor(
                            out=Scols[:, 3, :], in0=t4, in1=tcx, op=ADD)

                for m in range(nmt):
                    # ---- x_fin' = h'_{i-1} + S_i (per subtile) ----
                    if i == 0:
                        x_fin = xh[m][:, 0, :, :]
                    else:
                        xf = fins.tile([128, ST, D], BF16, tag="fin")
                        for s in range(ST):
                            scol = Scols[:, i, m * ST + s:m * ST + s + 1]
                            if SHIFT_ACT:
                                nc.scalar.activation(
                                    xf[:, s, :], xh[m][:, i, s, :],
                                    AF.Identity, bias=scol, scale=1.0)
                            else:
                                nc.vector.tensor_scalar(
                                    out=xf[:, s, :], in0=xh[m][:, i, s, :],
                                    scalar1=scol, scalar2=None, op0=ADD)
                        x_fin = xf[:]

                    # ---- transposes -> xT (stationary operands) ----
                    xT = xts.tile([128, ST, NCH, 128], BF16, tag="xT")
                    if TRANSP == "xbar":
                        for s in range(ST):
                            for c in range(NCH):
                                nc.sync.dma_start_transpose(
                                    xT[:, s, c, :],
                                    x_fin[:, s, c * 128:(c + 1) * 128])
                    else:
                        tr = tpsum.tile([128, ST, NCH, 128], BF16, tag="tr")
                        for s in range(ST):
                            for c in range(NCH):
                                nc.tensor.transpose(
                                    tr[:, s, c, :],
                                    x_fin[:, s, c * 128:(c + 1) * 128],
                                    ident[:])
                        if TRANSP == "pe_dve":
                            nc.vector.tensor_copy(xT[:], tr[:])
                        else:
                            nc.scalar.activation(xT[:], tr[:], AF.Copy)

                    # ---- matmuls ----
                    y_ps = ypsum.tile([128, ST, D], F32, tag="y")
                    nex = N_EX[i]
                    if nex:
                        e_ps = epsum.tile([128, ST, 16], F32, tag="e")
                    for s in range(ST):
                        for c in range(NCH):
                            nc.tensor.matmul(
                                y_ps[:, s, :], xT[:, s, c, :],
                                wt_sb[:, i, c, :],
                                start=(c == 0), stop=(c == NCH - 1))
                    if nex:
                        for s in range(ST):
                            for c in range(NCH):
                                nc.tensor.matmul(
                                    e_ps[:, s, :nex], xT[:, s, c, :],
                                    ex_sb[:, c, EX_OFF[i]:EX_OFF[i] + nex],
                                    start=(c == 0), stop=(c == NCH - 1))

                    # ---- evictions ----
                    nc.scalar.activation(xh[m][:, i + 1, :, :], y_ps[:],
                                         AF.Copy)
                    if nex:
                        nc.scalar.activation(
                            Ecols[:, EX_OFF[i]:EX_OFF[i] + nex,
                                  m * ST:(m + 1) * ST],
                            e_ps[:, :, :nex].rearrange("p s k -> p k s"),
                            AF.Copy)

            # ---- DMA out ----
            out_dram_ld = OUT.rearrange("(m s p) (l d) -> m s p l d",
                                        s=ST, p=128, l=NUM_LAYERS)
            for m in range(nmt):
                for s in range(ST):
                    nc.sync.dma_start(
                        out_dram_ld[m, s, :, :, :], xh[m][:, 1:, s, :])

    nc.compile()
    return nc


def _host_prep(W, b):
    """Fold biases into fixed vectors; build bf16 weight/extras operands."""
    import ml_dtypes
    W64 = np.asarray(W, np.float64)
    b64 = np.asarray(b, np.float64)
    Wt = W64.transpose(0, 2, 1)                      # [L, d, e]: y = x@Wt + b
    ones = np.ones(D)
    c = []
    prev = np.zeros(D)
    for i in range(NUM_LAYERS):
        ci = prev @ Wt[i] + b64[i]
        c.append(ci)
        prev = ci
    ex_cols = [c[0], c[1], c[2], Wt[0] @ c[1], Wt[0] @ c[2], Wt[0] @ ones,
               Wt[1] @ c[0], Wt[1] @ c[2], Wt[1] @ ones,
               Wt[2] @ c[0], Wt[2] @ c[1]]
    ex = np.stack(ex_cols, axis=1)                   # [512, 11]
    consts = (float(np.sum(c[0])), float(np.sum(c[1])),
              float(c[1] @ c[0]), float(c[2] @ c[0]), float(c[2] @ c[1]))
    bf = ml_dtypes.bfloat16
    wt_bf = np.ascontiguousarray(Wt.astype(np.float32)).astype(bf)
    ex_bf = np.ascontiguousarray(ex.astype(np.float32)).astype(bf)
    c_f32 = np.stack([ci.astype(np.float32) for ci in c], axis=0)  # [L, 512]
    return wt_bf, ex_bf, consts, c_f32


def run_shards(x, W, b, **spmd_kwargs):
    """Run the SPMD kernel; returns (full_output, BassKernelResults)."""
    import ml_dtypes
    from concourse.bass_utils import run_bass_kernel_spmd

    x_bf = np.ascontiguousarray(
        np.asarray(x, np.float32).astype(ml_dtypes.bfloat16))
    wt_bf, ex_bf, consts, c_f32 = _host_prep(np.asarray(W, np.float32),
                                             np.asarray(b, np.float32))

    if "nc" not in _CACHE:
        _CACHE["nc"] = _build_nc(consts=consts)
    nc = _CACHE["nc"]

    in_maps = []
    for cid in range(N_CORES):
        shard = x_bf[cid * ROWS_PER_CORE:(cid + 1) * ROWS_PER_CORE]
        in_maps.append({"x": np.ascontiguousarray(shard),
                        "wt": wt_bf, "ex": ex_bf})

    res = run_bass_kernel_spmd(nc, in_maps, core_ids=list(range(N_CORES)),
                               **spmd_kwargs)
    out = np.concatenate([r["out"] for r in res.results], axis=0)
    out = out.astype(np.float32)
    out += np.reshape(c_f32, (1, NUM_LAYERS * D))     # host bias fold-back
    return out, res


def kernel(x, W, b):
    out, _ = run_shards(x, W, b)
    return out


# revision 26
# speedup vs baseline: 4.9844x; 1.0397x over previous
"""Trainium2 Bass kernel for nn_CrossNetwork: 4-layer cross-network.

Reference semantics (per row b of x [B, D], D=512, L=4):
    x_list = [x]
    for i in range(L):
        h = x_list[-1]
        for p in x_list[:-1]:
            s = <h_cur, p>; h_cur += s        # sequential scalar residuals
        y = h_cur @ W[i].T + b[i]
        x_list.append(y)
    out = concat(x_list[1:])

Restructure (exact algebra):
 1. Scalar-shift recurrence: with D_j = <h, p_j> (h = raw layer input) and
    sig_j = rowsum(p_j):  S = sum_j s'_j,  s'_j = D_j + S_{<j} * sig_j.
 2. Bias elimination: only bias-free h'_i = x_fin'_i @ Wt_i live on chip,
    x_fin'_i = h'_{i-1} + S_i; the true y_i = h'_i + c_i with fixed
    c_i = c_{i-1} @ Wt_i + b_i (host adds c_i to the output).  All cross
    terms <h'_a, c_m> / rowsum(h'_a) are matmuls against fixed vectors ->
    a few extra moving columns sharing the main matmul's stationary.

Schedule: layer-major over 8 macro-tiles of 256 rows (2 subtiles x 128
partitions).  All per-row-scalar algebra is batched into [128, 16] ops
(one column per subtile).  Dots: one bf16 tensor_tensor multiply per
(layer, macro-tile) with the h operand broadcast over priors + one
segmented tensor_reduce (axis=X).  On-chip dtype bf16 except PSUM / the
per-row scalars (f32).  Output written bf16; host upcasts and adds c_i.
Sharding: batch split across 8 NeuronCores (data parallel, SPMD).
"""

import os
import numpy as np

NUM_LAYERS = 4
D = 512
B = 16384
N_CORES = 8
ROWS_PER_CORE = B // N_CORES          # 2048
ST = 2                                # subtiles per macro-tile
NMT = ROWS_PER_CORE // (128 * ST)     # 8 macro-tiles
NCH = D // 128                        # 4 contraction chunks

N_EX = (6, 3, 2, 0)                   # extras columns per layer
EX_OFF = (0, 6, 9, 11)
N_EX_TOT = 11

SHIFT_ACT = bool(int(os.environ.get("K_SHIFT_ACT", "1")))   # shifts on ACT
TRANSP = os.environ.get("K_TRANSP", "pe_dve")  # xbar | pe_act | pe_dve
RED_BF16 = bool(int(os.environ.get("K_RED_BF16", "0")))     # reduce out bf16
RED_MODE = os.environ.get("K_RED_MODE", "tr")  # tr | tsacc

_CACHE = {}


def _build_nc(nmt=NMT, consts=None):
    import concourse.tile as tile
    from concourse import bacc, mybir
    from concourse.masks import make_identity

    F32 = mybir.dt.float32
    BF16 = mybir.dt.bfloat16
    AF = mybir.ActivationFunctionType
    MUL = mybir.AluOpType.mult
    ADD = mybir.AluOpType.add

    csum0, csum1, K10, K20, K21 = consts
    rows = nmt * ST * 128

    nc = bacc.Bacc("TRN2", target_bir_lowering=False, debug=False)

    X = nc.dram_tensor("x", [rows, D], BF16, kind="ExternalInput")
    WT = nc.dram_tensor("wt", [NUM_LAYERS, D, D], BF16, kind="ExternalInput")
    EX = nc.dram_tensor("ex", [D, N_EX_TOT], BF16, kind="ExternalInput")
    OUT = nc.dram_tensor("out", [rows, NUM_LAYERS * D], BF16,
                         kind="ExternalOutput")

    NG = 2
    gmts = [list(range(g * nmt // NG, (g + 1) * nmt // NG))
            for g in range(NG)]
    gw = ST * nmt // NG                   # scalar columns per group

    with tile.TileContext(nc) as tc:
        with (
            tc.tile_pool(name="consts", bufs=1) as cpool,
            tc.tile_pool(name="persist", bufs=1) as ppool,
            tc.tile_pool(name="fins", bufs=3) as fins,
            tc.tile_pool(name="xts", bufs=3) as xts,
            tc.tile_pool(name="waste", bufs=2) as waste,
            tc.tile_pool(name="dcols", bufs=2) as dpool,
            tc.tile_pool(name="tmps", bufs=24) as tmps,
            tc.tile_pool(name="ypsum", bufs=2, space="PSUM") as ypsum,
            tc.tile_pool(name="tpsum", bufs=2, space="PSUM") as tpsum,
            tc.tile_pool(name="epsum", bufs=2, space="PSUM") as epsum,
        ):
            # ---- constants ----
            wt_sb = cpool.tile([128, NUM_LAYERS, NCH, D], BF16)
            wt_dram = WT.rearrange("l (c p) e -> l c p e", p=128)
            for l in range(NUM_LAYERS):
                for c in range(NCH):
                    nc.sync.dma_start(wt_sb[:, l, c, :], wt_dram[l, c, :, :])
            ex_sb = cpool.tile([128, NCH, N_EX_TOT], BF16)
            ex_dram = EX.rearrange("(c p) n -> c p n", p=128)
            for c in range(NCH):
                nc.sync.dma_start(ex_sb[:, c, :], ex_dram[c, :, :])
            ident = cpool.tile([128, 128], BF16)
            make_identity(nc, ident[:])
            # batched per-row scalars, one column per subtile, per MT-group
            Ecols = [cpool.tile([128, N_EX_TOT, gw], F32, name=f"Ecols{g}")
                     for g in range(NG)]
            Scols = [cpool.tile([128, NUM_LAYERS, gw], F32, name=f"Scols{g}")
                     for g in range(NG)]

            x_dram = X.rearrange("(m s p) d -> m s p d", s=ST, p=128)

            # persistent activation slots: 0 = x, 1+i = h'_i
            xh = [ppool.tile([128, NUM_LAYERS + 1, ST, D], BF16, tag=f"xh{m}",
                             name=f"xh{m}")
                  for m in range(nmt)]
            for m in range(nmt):
                for s in range(ST):
                    nc.sync.dma_start(xh[m][:, 0, s, :], x_dram[m, s, :, :])

            def tt(name, in0, in1, op):
                o = tmps.tile([128, gw], F32, tag="tmp", name=name)
                nc.vector.tensor_tensor(out=o[:], in0=in0, in1=in1, op=op)
                return o[:]

            def ts_imm(name, in0, imm, op):
                o = tmps.tile([128, gw], F32, tag="tmp", name=name)
                nc.vector.tensor_scalar(out=o[:], in0=in0, scalar1=float(imm),
                                        scalar2=None, op0=op)
                return o[:]

            for i in range(NUM_LAYERS):
              for g in range(NG):
                mts = gmts[g]
                base_m = mts[0]
                # ---- dots + batched recurrence -> S_i (skip layer 0) ----
                if i > 0:
                    Dc = dpool.tile([128, i, gw], F32, tag=f"D{g}",
                                    name=f"Dc{g}")
                    for m in mts:
                        lc = (m - base_m) * ST
                        w = waste.tile([128, i, ST, D], BF16, tag="w",
                                       name="w")
                        h_b = xh[m][:, i:i + 1, :, :].broadcast_to(
                            (128, i, ST, D))
                        nc.vector.tensor_tensor(
                            out=w[:], in0=h_b, in1=xh[m][:, 0:i, :, :],
                            op=MUL)
                        if RED_MODE == "tsacc":
                            wf = waste.tile([128, i, ST, D], BF16, tag="wf",
                                            name="wf")
                            for j in range(i):
                                for s in range(ST):
                                    nc.vector.tensor_scalar(
                                        out=wf[:, j, s, :], in0=w[:, j, s, :],
                                        scalar1=0.0, scalar2=None,
                                        op0=ADD, op1=ADD,
                                        accum_out=Dc[:, j, lc + s:lc + s + 1])
                        else:
                            nc.vector.tensor_reduce(
                                out=Dc[:, :, lc:lc + ST], in_=w[:],
                                axis=mybir.AxisListType.X, op=ADD)
                    E = lambda k: Ecols[g][:, k, :]
                    Dr = lambda j: Dc[:, j, :]
                    Sdst = Scols[g][:, i, :]
                    if i == 1:
                        nc.vector.tensor_tensor(
                            out=Sdst, in0=Dr(0), in1=E(0), op=ADD)
                    elif i == 2:
                        Da = tt("Da", Dr(0), E(1), ADD)
                        pre = tt("pre", E(6), E(3), ADD)
                        pre = ts_imm("preK", pre, K10, ADD)
                        Db = tt("Db", Dr(1), pre, ADD)
                        ta = ts_imm("ta", Da, 1.0 + csum0, MUL)
                        ta = tt("ta2", ta, Db, ADD)
                        t3 = tt("t3", E(5), Da, MUL)
                        nc.vector.tensor_tensor(
                            out=Sdst, in0=t3, in1=ta, op=ADD)
                    else:
                        Da = tt("Da", Dr(0), E(2), ADD)
                        p1 = tt("p1", E(9), E(4), ADD)
                        p1 = ts_imm("p1K", p1, K20, ADD)
                        Db = tt("Db", Dr(1), p1, ADD)
                        p2 = tt("p2", E(10), E(7), ADD)
                        p2 = ts_imm("p2K", p2, K21, ADD)
                        Dck = tt("Dck", Dr(2), p2, ADD)
                        ta = ts_imm("ta", Da, 1.0 + csum0, MUL)
                        ta = tt("ta2", ta, Db, ADD)
                        t3 = tt("t3", E(5), Da, MUL)
                        b_ = tt("b", t3, ta, ADD)
                        tcx = ts_imm("tc", b_, 1.0 + csum1, MUL)
                        tcx = tt("tc2", tcx, Dck, ADD)
                        t4 = tt("t4", E(8), b_, MUL)
                        nc.vector.tensor_tensor(
                            out=Sdst, in0=t4, in1=tcx, op=ADD)

                for m in mts:
                    lc = (m - base_m) * ST
                    # ---- x_fin' = h'_{i-1} + S_i (per subtile) ----
                    if i == 0:
                        x_fin = xh[m][:, 0, :, :]
                    else:
                        xf = fins.tile([128, ST, D], BF16, tag="fin",
                                       name="xf")
                        for s in range(ST):
                            scol = Scols[g][:, i, lc + s:lc + s + 1]
                            if SHIFT_ACT:
                                nc.scalar.activation(
                                    xf[:, s, :], xh[m][:, i, s, :],
                                    AF.Identity, bias=scol, scale=1.0)
                            else:
                                nc.vector.tensor_scalar(
                                    out=xf[:, s, :], in0=xh[m][:, i, s, :],
                                    scalar1=scol, scalar2=None, op0=ADD)
                        x_fin = xf[:]

                    # ---- transposes -> xT (stationary operands) ----
                    xT = xts.tile([128, ST, NCH, 128], BF16, tag="xT",
                                  name="xT")
                    if TRANSP == "xbar":
                        for s in range(ST):
                            for c in range(NCH):
                                nc.sync.dma_start_transpose(
                                    xT[:, s, c, :],
                                    x_fin[:, s, c * 128:(c + 1) * 128])
                    else:
                        tr = tpsum.tile([128, ST, NCH, 128], BF16, tag="tr",
                                        name="tr")
                        for s in range(ST):
                            for c in range(NCH):
                                nc.tensor.transpose(
                                    tr[:, s, c, :],
                                    x_fin[:, s, c * 128:(c + 1) * 128],
                                    ident[:])
                        if TRANSP == "pe_dve":
                            nc.vector.tensor_copy(xT[:], tr[:])
                        else:
                            nc.scalar.activation(xT[:], tr[:], AF.Copy)

                    # ---- matmuls ----
                    y_ps = ypsum.tile([128, ST, D], F32, tag="y", name="y_ps")
                    nex = N_EX[i]
                    if nex:
                        e_ps = epsum.tile([128, ST, 16], F32, tag="e",
                                          name="e_ps")
                    for s in range(ST):
                        for c in range(NCH):
                            nc.tensor.matmul(
                                y_ps[:, s, :], xT[:, s, c, :],
                                wt_sb[:, i, c, :],
                                start=(c == 0), stop=(c == NCH - 1))
                    if nex:
                        for s in range(ST):
                            for c in range(NCH):
                                nc.tensor.matmul(
                                    e_ps[:, s, :nex], xT[:, s, c, :],
                                    ex_sb[:, c, EX_OFF[i]:EX_OFF[i] + nex],
                                    start=(c == 0), stop=(c == NCH - 1))

                    # ---- evictions ----
                    nc.scalar.activation(xh[m][:, i + 1, :, :], y_ps[:],
                                         AF.Copy)
                    if nex:
                        nc.scalar.activation(
                            Ecols[g][:, EX_OFF[i]:EX_OFF[i] + nex,
                                     lc:lc + ST],
                            e_ps[:, :, :nex].rearrange("p s k -> p k s"),
                            AF.Copy)

            # ---- DMA out ----
            out_dram_ld = OUT.rearrange("(m s p) (l d) -> m s p l d",
                                        s=ST, p=128, l=NUM_LAYERS)
            for m in range(nmt):
                for s in range(ST):
                    nc.sync.dma_start(
                        out_dram_ld[m, s, :, :, :], xh[m][:, 1:, s, :])

    nc.compile()
    return nc


def _host_prep(W, b):
    """Fold biases into fixed vectors; build bf16 weight/extras operands."""
    import ml_dtypes
    W64 = np.asarray(W, np.float64)
    b64 = np.asarray(b, np.float64)
    Wt = W64.transpose(0, 2, 1)                      # [L, d, e]: y = x@Wt + b
    ones = np.ones(D)
    c = []
    prev = np.zeros(D)
    for i in range(NUM_LAYERS):
        ci = prev @ Wt[i] + b64[i]
        c.append(ci)
        prev = ci
    ex_cols = [c[0], c[1], c[2], Wt[0] @ c[1], Wt[0] @ c[2], Wt[0] @ ones,
               Wt[1] @ c[0], Wt[1] @ c[2], Wt[1] @ ones,
               Wt[2] @ c[0], Wt[2] @ c[1]]
    ex = np.stack(ex_cols, axis=1)                   # [512, 11]
    consts = (float(np.sum(c[0])), float(np.sum(c[1])),
              float(c[1] @ c[0]), float(c[2] @ c[0]), float(c[2] @ c[1]))
    bf = ml_dtypes.bfloat16
    wt_bf = np.ascontiguousarray(Wt.astype(np.float32)).astype(bf)
    ex_bf = np.ascontiguousarray(ex.astype(np.float32)).astype(bf)
    c_f32 = np.stack([ci.astype(np.float32) for ci in c], axis=0)  # [L, 512]
    return wt_bf, ex_bf, consts, c_f32


def run_shards(x, W, b, **spmd_kwargs):
    """Run the SPMD kernel; returns (full_output, BassKernelResults)."""
    import ml_dtypes
    from concourse.bass_utils import run_bass_kernel_spmd

    x_bf = np.ascontiguousarray(
        np.asarray(x, np.float32).astype(ml_dtypes.bfloat16))
    wt_bf, ex_bf, consts, c_f32 = _host_prep(np.asarray(W, np.float32),
                                             np.asarray(b, np.float32))

    if "nc" not in _CACHE:
        _CACHE["nc"] = _build_nc(consts=consts)
    nc = _CACHE["nc"]

    in_maps = []
    for cid in range(N_CORES):
        shard = x_bf[cid * ROWS_PER_CORE:(cid + 1) * ROWS_PER_CORE]
        in_maps.append({"x": np.ascontiguousarray(shard),
                        "wt": wt_bf, "ex": ex_bf})

    res = run_bass_kernel_spmd(nc, in_maps, core_ids=list(range(N_CORES)),
                               **spmd_kwargs)
    out = np.concatenate([r["out"] for r in res.results], axis=0)
    out = out.astype(np.float32)
    out += np.reshape(c_f32, (1, NUM_LAYERS * D))     # host bias fold-back
    return out, res


def kernel(x, W, b):
    out, _ = run_shards(x, W, b)
    return out


# revision 28
# speedup vs baseline: 5.8491x; 1.1735x over previous
"""Trainium2 Bass kernel for nn_CrossNetwork: 4-layer cross-network.

Reference semantics (per row b of x [B, D], D=512, L=4):
    x_list = [x]
    for i in range(L):
        h = x_list[-1]
        for p in x_list[:-1]:
            s = <h_cur, p>; h_cur += s        # sequential scalar residuals
        y = h_cur @ W[i].T + b[i]
        x_list.append(y)
    out = concat(x_list[1:])

Restructure (exact algebra):
 1. Scalar-shift recurrence: with D_j = <h, p_j> (h = raw layer input) and
    sig_j = rowsum(p_j):  S = sum_j s'_j,  s'_j = D_j + S_{<j} * sig_j.
 2. Bias elimination: only bias-free h'_i = x_fin'_i @ Wt_i live on chip,
    x_fin'_i = h'_{i-1} + S_i; the true y_i = h'_i + c_i with fixed
    c_i = c_{i-1} @ Wt_i + b_i (host adds c_i to the output).  All cross
    terms <h'_a, c_m> / rowsum(h'_a) are matmuls against fixed vectors ->
    a few extra moving columns sharing the main matmul's stationary.

Schedule: layer-major over 8 macro-tiles of 256 rows (2 subtiles x 128
partitions).  All per-row-scalar algebra is batched into [128, 16] ops
(one column per subtile).  Dots: one bf16 tensor_tensor multiply per
(layer, macro-tile) with the h operand broadcast over priors + one
segmented tensor_reduce (axis=X).  On-chip dtype bf16 except PSUM / the
per-row scalars (f32).  Output written bf16; host upcasts and adds c_i.
Sharding: batch split across 8 NeuronCores (data parallel, SPMD).
"""

import os
import numpy as np

NUM_LAYERS = 4
D = 512
B = 16384
N_CORES = 8
ROWS_PER_CORE = B // N_CORES          # 2048
ST = 2                                # subtiles per macro-tile
NMT = ROWS_PER_CORE // (128 * ST)     # 8 macro-tiles
NCH = D // 128                        # 4 contraction chunks

N_EX = (6, 3, 2, 0)                   # extras columns per layer
EX_OFF = (0, 6, 9, 11)
N_EX_TOT = 11

SHIFT_ACT = bool(int(os.environ.get("K_SHIFT_ACT", "1")))   # shifts on ACT
TRANSP = os.environ.get("K_TRANSP", "pe_dve")  # xbar | pe_act | pe_dve
RED_BF16 = bool(int(os.environ.get("K_RED_BF16", "0")))     # reduce out bf16
RED_MODE = os.environ.get("K_RED_MODE", "stt")  # tr | tsacc | stt

_CACHE = {}


def _build_nc(nmt=NMT, consts=None):
    import concourse.tile as tile
    from concourse import bacc, mybir
    from concourse.masks import make_identity

    F32 = mybir.dt.float32
    BF16 = mybir.dt.bfloat16
    AF = mybir.ActivationFunctionType
    MUL = mybir.AluOpType.mult
    ADD = mybir.AluOpType.add

    csum0, csum1, K10, K20, K21 = consts
    rows = nmt * ST * 128

    nc = bacc.Bacc("TRN2", target_bir_lowering=False, debug=False)

    X = nc.dram_tensor("x", [rows, D], BF16, kind="ExternalInput")
    WT = nc.dram_tensor("wt", [NUM_LAYERS, D, D], BF16, kind="ExternalInput")
    EX = nc.dram_tensor("ex", [D, N_EX_TOT], BF16, kind="ExternalInput")
    OUT = nc.dram_tensor("out", [rows, NUM_LAYERS * D], BF16,
                         kind="ExternalOutput")

    NG = 2
    gmts = [list(range(g * nmt // NG, (g + 1) * nmt // NG))
            for g in range(NG)]
    gw = ST * nmt // NG                   # scalar columns per group

    with tile.TileContext(nc) as tc:
        with (
            tc.tile_pool(name="consts", bufs=1) as cpool,
            tc.tile_pool(name="persist", bufs=1) as ppool,
            tc.tile_pool(name="fins", bufs=3) as fins,
            tc.tile_pool(name="xts", bufs=3) as xts,
            tc.tile_pool(name="waste", bufs=2) as waste,
            tc.tile_pool(name="dcols", bufs=2) as dpool,
            tc.tile_pool(name="tmps", bufs=24) as tmps,
            tc.tile_pool(name="ypsum", bufs=2, space="PSUM") as ypsum,
            tc.tile_pool(name="tpsum", bufs=2, space="PSUM") as tpsum,
            tc.tile_pool(name="epsum", bufs=2, space="PSUM") as epsum,
        ):
            # ---- persistent activations; slot 0 = x, 1+i = h'_i ----
            x_dram = X.rearrange("(m s p) d -> m p s d", s=ST, p=128)
            xh = [ppool.tile([128, ST, NUM_LAYERS + 1, D], BF16, tag=f"xh{m}",
                             name=f"xh{m}")
                  for m in range(nmt)]
            for m in range(nmt):
                nc.sync.dma_start(xh[m][:, :, 0, :], x_dram[m, :, :, :])

            # ---- constants (input x first: layer 0 starts sooner) ----
            wt_sb = cpool.tile([128, NUM_LAYERS, NCH, D], BF16)
            wt_dram = WT.rearrange("l (c p) e -> l p c e", p=128)
            for l in range(NUM_LAYERS):
                nc.scalar.dma_start(wt_sb[:, l, :, :], wt_dram[l, :, :, :])
            ex_sb = cpool.tile([128, NCH, N_EX_TOT], BF16)
            ex_dram = EX.rearrange("(c p) n -> p c n", p=128)
            nc.scalar.dma_start(ex_sb[:, :, :], ex_dram[:, :, :])
            ident = cpool.tile([128, 128], BF16)
            make_identity(nc, ident[:])
            out_dram_ld = OUT.rearrange("(m s p) (l d) -> m s p l d",
                                        s=ST, p=128, l=NUM_LAYERS)
            # batched per-row scalars, one column per subtile, per MT-group
            Ecols = [cpool.tile([128, N_EX_TOT, gw], F32, name=f"Ecols{g}")
                     for g in range(NG)]
            Scols = [cpool.tile([128, NUM_LAYERS, gw], F32, name=f"Scols{g}")
                     for g in range(NG)]

            def tt(name, in0, in1, op):
                o = tmps.tile([128, gw], F32, tag="tmp", name=name)
                nc.vector.tensor_tensor(out=o[:], in0=in0, in1=in1, op=op)
                return o[:]

            def ts_imm(name, in0, imm, op):
                o = tmps.tile([128, gw], F32, tag="tmp", name=name)
                nc.vector.tensor_scalar(out=o[:], in0=in0, scalar1=float(imm),
                                        scalar2=None, op0=op)
                return o[:]

            for i in range(NUM_LAYERS):
              Dcs = {}
              for g in range(NG):
                mts = gmts[g]
                base_m = mts[0]
                # ---- dots + batched recurrence -> S_i (skip layer 0) ----
                if i > 0:
                    Dc = dpool.tile([128, i, gw], F32, tag=f"D{g}",
                                    name=f"Dc{g}")
                    Dcs[g] = Dc
                    for m in mts:
                        lc = (m - base_m) * ST
                        w = waste.tile([128, ST, i, D], BF16, tag="w",
                                       name="w")
                        if RED_MODE == "stt":
                            for s in range(ST):
                                for j in range(i):
                                    nc.vector.scalar_tensor_tensor(
                                        out=w[:, s, j, :],
                                        in0=xh[m][:, s, i, :], scalar=1.0,
                                        in1=xh[m][:, s, j, :],
                                        op0=MUL, op1=MUL,
                                        accum_out=Dc[:, j, lc + s:lc + s + 1])
                        else:
                            h_b = xh[m][:, :, i:i + 1, :].broadcast_to(
                                (128, ST, i, D))
                            nc.vector.tensor_tensor(
                                out=w[:], in0=h_b, in1=xh[m][:, :, 0:i, :],
                                op=MUL)
                            nc.vector.tensor_reduce(
                                out=Dc[:, 0:i, lc:lc + ST].rearrange(
                                    "p j s -> p s j"),
                                in_=w[:], axis=mybir.AxisListType.X, op=ADD)
                    E = lambda k: Ecols[g][:, k, :]
                    Dr = lambda j: Dc[:, j, :]
                    Sdst = Scols[g][:, i, :]
                    if i == 1:
                        nc.vector.tensor_tensor(
                            out=Sdst, in0=Dr(0), in1=E(0), op=ADD)
                    elif i == 2:
                        Da = tt("Da", Dr(0), E(1), ADD)
                        pre = tt("pre", E(6), E(3), ADD)
                        pre = ts_imm("preK", pre, K10, ADD)
                        Db = tt("Db", Dr(1), pre, ADD)
                        ta = ts_imm("ta", Da, 1.0 + csum0, MUL)
                        ta = tt("ta2", ta, Db, ADD)
                        t3 = tt("t3", E(5), Da, MUL)
                        nc.vector.tensor_tensor(
                            out=Sdst, in0=t3, in1=ta, op=ADD)
                    else:
                        Da = tt("Da", Dr(0), E(2), ADD)
                        p1 = tt("p1", E(9), E(4), ADD)
                        p1 = ts_imm("p1K", p1, K20, ADD)
                        Db = tt("Db", Dr(1), p1, ADD)
                        p2 = tt("p2", E(10), E(7), ADD)
                        p2 = ts_imm("p2K", p2, K21, ADD)
                        Dck = tt("Dck", Dr(2), p2, ADD)
                        ta = ts_imm("ta", Da, 1.0 + csum0, MUL)
                        ta = tt("ta2", ta, Db, ADD)
                        t3 = tt("t3", E(5), Da, MUL)
                        b_ = tt("b", t3, ta, ADD)
                        tcx = ts_imm("tc", b_, 1.0 + csum1, MUL)
                        tcx = tt("tc2", tcx, Dck, ADD)
                        t4 = tt("t4", E(8), b_, MUL)
                        nc.vector.tensor_tensor(
                            out=Sdst, in0=t4, in1=tcx, op=ADD)

              for g in range(NG):
                mts = gmts[g]
                base_m = mts[0]
                for m in mts:
                    lc = (m - base_m) * ST
                    # ---- x_fin' = h'_{i-1} + S_i (per subtile) ----
                    if i == 0:
                        x_fin = xh[m][:, :, 0, :]
                    else:
                        xf = fins.tile([128, ST, D], BF16, tag="fin",
                                       name="xf")
                        for s in range(ST):
                            scol = Scols[g][:, i, lc + s:lc + s + 1]
                            if SHIFT_ACT:
                                nc.scalar.activation(
                                    xf[:, s, :], xh[m][:, s, i, :],
                                    AF.Identity, bias=scol, scale=1.0)
                            else:
                                nc.vector.tensor_scalar(
                                    out=xf[:, s, :], in0=xh[m][:, s, i, :],
                                    scalar1=scol, scalar2=None, op0=ADD)
                        x_fin = xf[:]

                    # ---- transposes -> xT (stationary operands) ----
                    xT = xts.tile([128, ST, NCH, 128], BF16, tag="xT",
                                  name="xT")
                    if TRANSP == "xbar":
                        for s in range(ST):
                            for c in range(NCH):
                                nc.sync.dma_start_transpose(
                                    xT[:, s, c, :],
                                    x_fin[:, s, c * 128:(c + 1) * 128])
                    else:
                        tr = tpsum.tile([128, ST, NCH, 128], BF16, tag="tr",
                                        name="tr")
                        for s in range(ST):
                            for c in range(NCH):
                                nc.tensor.transpose(
                                    tr[:, s, c, :],
                                    x_fin[:, s, c * 128:(c + 1) * 128],
                                    ident[:])
                        if TRANSP == "pe_dve":
                            nc.vector.tensor_copy(xT[:], tr[:])
                        else:
                            nc.scalar.activation(xT[:], tr[:], AF.Copy)

                    # ---- matmuls ----
                    y_ps = ypsum.tile([128, ST, D], F32, tag="y", name="y_ps")
                    nex = N_EX[i]
                    if nex:
                        e_ps = epsum.tile([128, ST, 16], F32, tag="e",
                                          name="e_ps")
                    for s in range(ST):
                        for c in range(NCH):
                            nc.tensor.matmul(
                                y_ps[:, s, :], xT[:, s, c, :],
                                wt_sb[:, i, c, :],
                                start=(c == 0), stop=(c == NCH - 1))
                    if nex:
                        for s in range(ST):
                            for c in range(NCH):
                                nc.tensor.matmul(
                                    e_ps[:, s, :nex], xT[:, s, c, :],
                                    ex_sb[:, c, EX_OFF[i]:EX_OFF[i] + nex],
                                    start=(c == 0), stop=(c == NCH - 1))

                    # ---- evictions ----
                    nc.scalar.activation(xh[m][:, :, i + 1, :], y_ps[:],
                                         AF.Copy)
                    if nex:
                        nc.scalar.activation(
                            Ecols[g][:, EX_OFF[i]:EX_OFF[i] + nex,
                                     lc:lc + ST],
                            e_ps[:, :, :nex].rearrange("p s k -> p k s"),
                            AF.Copy)
                    if i == NUM_LAYERS - 1:
                        eng = nc.sync if m % 2 == 0 else nc.scalar
                        for s in range(ST):
                            eng.dma_start(out_dram_ld[m, s, :, :, :],
                                          xh[m][:, s, 1:, :])

    nc.compile()
    return nc


def _host_prep(W, b):
    """Fold biases into fixed vectors; build bf16 weight/extras operands."""
    import ml_dtypes
    W64 = np.asarray(W, np.float64)
    b64 = np.asarray(b, np.float64)
    Wt = W64.transpose(0, 2, 1)                      # [L, d, e]: y = x@Wt + b
    ones = np.ones(D)
    c = []
    prev = np.zeros(D)
    for i in range(NUM_LAYERS):
        ci = prev @ Wt[i] + b64[i]
        c.append(ci)
        prev = ci
    ex_cols = [c[0], c[1], c[2], Wt[0] @ c[1], Wt[0] @ c[2], Wt[0] @ ones,
               Wt[1] @ c[0], Wt[1] @ c[2], Wt[1] @ ones,
               Wt[2] @ c[0], Wt[2] @ c[1]]
    ex = np.stack(ex_cols, axis=1)                   # [512, 11]
    consts = (float(np.sum(c[0])), float(np.sum(c[1])),
              float(c[1] @ c[0]), float(c[2] @ c[0]), float(c[2] @ c[1]))
    bf = ml_dtypes.bfloat16
    wt_bf = np.ascontiguousarray(Wt.astype(np.float32)).astype(bf)
    ex_bf = np.ascontiguousarray(ex.astype(np.float32)).astype(bf)
    c_f32 = np.stack([ci.astype(np.float32) for ci in c], axis=0)  # [L, 512]
    return wt_bf, ex_bf, consts, c_f32


def run_shards(x, W, b, **spmd_kwargs):
    """Run the SPMD kernel; returns (full_output, BassKernelResults)."""
    import ml_dtypes
    from concourse.bass_utils import run_bass_kernel_spmd

    x_bf = np.ascontiguousarray(
        np.asarray(x, np.float32).astype(ml_dtypes.bfloat16))
    wt_bf, ex_bf, consts, c_f32 = _host_prep(np.asarray(W, np.float32),
                                             np.asarray(b, np.float32))

    if "nc" not in _CACHE:
        _CACHE["nc"] = _build_nc(consts=consts)
    nc = _CACHE["nc"]

    in_maps = []
    for cid in range(N_CORES):
        shard = x_bf[cid * ROWS_PER_CORE:(cid + 1) * ROWS_PER_CORE]
        in_maps.append({"x": np.ascontiguousarray(shard),
                        "wt": wt_bf, "ex": ex_bf})

    res = run_bass_kernel_spmd(nc, in_maps, core_ids=list(range(N_CORES)),
                               **spmd_kwargs)
    out = np.concatenate([r["out"] for r in res.results], axis=0)
    out = out.astype(np.float32)
    out += np.reshape(c_f32, (1, NUM_LAYERS * D))     # host bias fold-back
    return out, res


def kernel(x, W, b):
    out, _ = run_shards(x, W, b)
    return out
